# revision 16
# baseline (speedup 1.0000x reference)
"""Trainium2 Bass kernel for a batched LSTM (B=128, S=1024, I=H=128).

Strategy (data-parallel over batch across 8 NeuronCores, 16 seqs/core):
  - Phase 1 (parallel GEMM): xp[s,b,g] = x @ W_ih^T + (b_ih + b_hh), computed
    gate-block-major so gates land on partitions; stored bf16 in SBUF.
  - Recurrence: 1024 serial steps; per step 4 small matmuls accumulate
    W_hh @ h_{t-1} in PSUM on top of pre-staged xp (has_written trick),
    ACT does sigmoid/tanh straight from PSUM, DVE does the products.
    All state kept transposed ([H partitions, batch]) so elementwise ops are
    cheap and no transposes appear in the serial loop.
  - Outputs are collected transposed and flipped back to [b, s, h] with PE
    transposes every 8 steps, then DMAed out.
Matmul operands in bf16 (fp32 PSUM accumulate); elementwise tail in fp32.
"""

import numpy as np
from contextlib import ExitStack

import concourse.bass as bass
import concourse.bacc as bacc
import concourse.tile as tile
from concourse import mybir
from concourse.bass_utils import run_bass_kernel_spmd
from concourse.masks import make_identity

B, S, I, H = 128, 1024, 128, 128
NCORES = 8
BS = B // NCORES            # 16 sequences per core
F32 = mybir.dt.float32
BF16 = mybir.dt.bfloat16
AF = mybir.ActivationFunctionType
# gate order used on-chip: [i, f, o, g]; reference row-block order is [i, f, g, o]
REFIDX = [0, 1, 3, 2]

U = 128                     # steps per For_i body
NITER = S // U


def _build_lstm(nc: bass.Bass, tc: tile.TileContext, ctx: ExitStack,
                x_in, wih_in, whh_in, bih_in, bhh_in, out_o, h_o, c_o,
                n_steps: int):
    u_steps = n_steps
    n_grp = n_steps // 32

    consts = ctx.enter_context(tc.tile_pool(name="consts", bufs=1))
    xnat_pool = ctx.enter_context(tc.tile_pool(name="xnat", bufs=3))
    xt_pool = ctx.enter_context(tc.tile_pool(name="xt", bufs=2))
    outsb_pool = ctx.enter_context(tc.tile_pool(name="outsb", bufs=3))
    tr_psum = ctx.enter_context(tc.tile_pool(name="trps", bufs=2, space="PSUM"))
    p1_psum = ctx.enter_context(tc.tile_pool(name="p1ps", bufs=2, space="PSUM"))

    ident_f = consts.tile([128, 128], F32)
    make_identity(nc, ident_f)
    ident_b = consts.tile([128, 128], BF16)
    make_identity(nc, ident_b)

    # ---- weights: W^T blocks (bf16), gate order [i, f, o, g] on columns ----
    wT_ih = consts.tile([128, 4 * H], BF16)
    wT_hh = consts.tile([128, 4 * H], BF16)
    for src, dstT in ((wih_in, wT_ih), (whh_in, wT_hh)):
        for g in range(4):
            r = REFIDX[g]
            w_nat = xnat_pool.tile([128, 128], F32, name="w_nat")
            nc.sync.dma_start(out=w_nat, in_=src[r * 128:(r + 1) * 128, :])
            wps = tr_psum.tile([128, 128], F32, name="wps", tag="trtile")
            nc.tensor.transpose(wps, w_nat, ident_f)
            nc.vector.tensor_copy(dstT[:, g * 128:(g + 1) * 128], wps)

    # ---- biases: [128, 4] in reference gate order; indexed via REFIDX ----
    b1 = consts.tile([128, 4], F32)
    b2 = consts.tile([128, 4], F32)
    nc.sync.dma_start(out=b1, in_=bih_in.rearrange("(g p) -> p g", p=128))
    nc.sync.dma_start(out=b2, in_=bhh_in.rearrange("(g p) -> p g", p=128))
    bias = consts.tile([128, 4], F32)
    nc.vector.tensor_add(bias, b1, b2)

    # ---- phase 1: xp[h', step, gate*16+b] bf16 (bias folded in) ----
    xp = consts.tile([128, n_steps, 4 * BS], BF16)
    for grp in range(n_grp):
        s0 = grp * 32
        xt = xt_pool.tile([128, 512], BF16)
        for j in range(4):
            x_nat = xnat_pool.tile([128, 128], F32)
            src = x_in[:, s0 + 8 * j: s0 + 8 * j + 8, :].transpose([1, 0, 2])
            nc.sync.dma_start(out=x_nat, in_=src)
            xps = tr_psum.tile([128, 128], F32, name="xps", tag="trtile")
            nc.tensor.transpose(xps, x_nat, ident_f)
            if j % 2 == 0:
                nc.vector.tensor_copy(xt[:, 128 * j:128 * (j + 1)], xps)
            else:
                nc.scalar.copy(xt[:, 128 * j:128 * (j + 1)], xps)
        for g in range(4):
            pp = p1_psum.tile([128, 512], F32, name="pp")
            nc.tensor.matmul(pp, lhsT=wT_ih[:, g * 128:(g + 1) * 128], rhs=xt,
                             start=True, stop=True)
            dst = xp[:, s0:s0 + 32, g * BS:(g + 1) * BS]
            src = pp.rearrange("p (s b) -> p s b", b=BS)
            bias_ap = bias[:, REFIDX[g]:REFIDX[g] + 1]
            if g % 2 == 0:
                nc.vector.tensor_scalar_add(dst, src, bias_ap)
            else:
                nc.scalar.add(dst, src, bias_ap)

    # ---- recurrence state (persistent tiles; distinct names = own buffers) ----
    rec_psum = ctx.enter_context(tc.tile_pool(name="recps", bufs=1, space="PSUM"))
    X = consts.tile([128, 32], F32, name="X")          # cols 0:16 tanh(g), 16:32 c
    s_ifo = consts.tile([128, 48], F32, name="s_ifo")  # sigmoid outputs [i | f | o]
    th = consts.tile([128, 16], F32, name="th")        # tanh(c)
    P = consts.tile([128, 32], F32, name="P")          # [i*tg | f*c]
    ob0 = consts.tile([128, 8 * BS], BF16, name="ob0")  # h slices, ping
    ob1 = consts.tile([128, 8 * BS], BF16, name="ob1")  # h slices, pong
    pA = rec_psum.tile([128, 64], F32, name="pA")
    pB = rec_psum.tile([128, 64], F32, name="pB")

    nc.vector.memset(X[:, 16:32], 0.0)          # c0 = 0
    nc.vector.memset(ob1[:, 112:128], 0.0)      # h0 = 0 (read by first step)
    # warm has_written bits in pA/pB so later start=False matmuls accumulate
    # onto DVE-staged xp (values are garbage; staging overwrites them)
    nc.tensor.matmul(pA, lhsT=wT_hh[:, 0:128], rhs=wT_hh[:, 0:64],
                     start=True, stop=True)
    nc.tensor.matmul(pB, lhsT=wT_hh[:, 0:128], rhs=wT_hh[:, 0:64],
                     start=True, stop=True)

    xp_v = xp.rearrange("p s c -> p (s c)")  # [128, n_steps*64]

    def step(u, hprev):
        """One LSTM step; returns the h slice written (matmul rhs of next)."""
        pX = pA if (u % 2 == 0) else pB
        ob = ob1 if ((u // 8) % 2 == 1) else ob0
        # stage xp for this step into PSUM (off critical path, 1 step ahead
        # thanks to pA/pB double buffering)
        nc.vector.tensor_copy(pX, xp_v[:, u * 4 * BS:(u + 1) * 4 * BS])
        # gates += W_hh^T-block @ h  (accumulate onto staged xp)
        for g in range(4):
            nc.tensor.matmul(pX[:, g * BS:(g + 1) * BS],
                             lhsT=wT_hh[:, g * 128:(g + 1) * 128],
                             rhs=hprev, start=False, stop=True,
                             skip_group_check=True)
        nc.scalar.activation(s_ifo, pX[:, 0:48], AF.Sigmoid)
        nc.scalar.activation(X[:, 0:16], pX[:, 48:64], AF.Tanh)
        nc.vector.tensor_mul(P, s_ifo[:, 0:32], X[:, 0:32])
        nc.vector.tensor_add(X[:, 16:32], P[:, 0:16], P[:, 16:32])   # c
        nc.scalar.activation(th, X[:, 16:32], AF.Tanh)
        hs = ob[:, (u % 8) * BS:(u % 8 + 1) * BS]
        nc.vector.tensor_mul(hs, s_ifo[:, 32:48], th)                # h (bf16)
        return hs, ob

    def flush_out(u, ob):
        """Transpose the last 8 steps' h back to [s,b,h] and DMA out."""
        pst = tr_psum.tile([128, 128], BF16, name="pst", tag="trtile")
        nc.tensor.transpose(pst, ob, ident_b)
        osb = outsb_pool.tile([128, 128], F32)
        nc.scalar.copy(osb, pst)
        s_off = (u // 8) * 8
        dst = out_o[:, s_off:s_off + 8, :].transpose([1, 0, 2])
        nc.sync.dma_start(out=dst, in_=osb)

    hprev = ob1[:, 112:128]
    for u in range(u_steps):
        hprev, ob = step(u, hprev)
        if u % 8 == 7:
            flush_out(u, ob)

    # ---- final h, c ----
    phs = tr_psum.tile([16, 128], BF16, name="phs", tag="trtile")
    nc.tensor.transpose(phs, ob1[:, 112:128], ident_b)
    h_sb = consts.tile([16, 128], F32, name="h_sb")
    nc.vector.tensor_copy(h_sb, phs)
    nc.sync.dma_start(out=h_o[:, :], in_=h_sb)

    pcs = tr_psum.tile([16, 128], F32, name="pcs", tag="trtile")
    nc.tensor.transpose(pcs, X[:, 16:32], ident_f)
    c_sb = consts.tile([16, 128], F32, name="c_sb")
    nc.vector.tensor_copy(c_sb, pcs)
    nc.sync.dma_start(out=c_o[:, :], in_=c_sb)


def build_nc(n_steps: int = S):
    nc = bacc.Bacc(None)
    x_in = nc.declare_dram_parameter("x", [BS, n_steps, I], F32, isOutput=False)
    wih_in = nc.declare_dram_parameter("W_ih", [4 * H, I], F32, isOutput=False)
    whh_in = nc.declare_dram_parameter("W_hh", [4 * H, H], F32, isOutput=False)
    bih_in = nc.declare_dram_parameter("b_ih", [4 * H], F32, isOutput=False)
    bhh_in = nc.declare_dram_parameter("b_hh", [4 * H], F32, isOutput=False)
    out_o = nc.declare_dram_parameter("out", [BS, n_steps, H], F32, isOutput=True)
    h_o = nc.declare_dram_parameter("h_out", [BS, H], F32, isOutput=True)
    c_o = nc.declare_dram_parameter("c_out", [BS, H], F32, isOutput=True)

    with tile.TileContext(nc) as tc, ExitStack() as ctx:
        _build_lstm(nc, tc, ctx, x_in[:], wih_in[:], whh_in[:], bih_in[:],
                    bhh_in[:], out_o[:], h_o[:], c_o[:], n_steps)
    nc.compile()
    return nc


_NC_CACHE = {}


def _get_nc(n_steps=S):
    if n_steps not in _NC_CACHE:
        _NC_CACHE[n_steps] = build_nc(n_steps)
    return _NC_CACHE[n_steps]


def kernel(x, W_ih, W_hh, b_ih, b_hh, _trace=False):
    # NOTE: trace requires antenv.axon_hooks (absent here); keep False.
    x = np.ascontiguousarray(np.asarray(x, dtype=np.float32))
    W_ih = np.ascontiguousarray(np.asarray(W_ih, dtype=np.float32))
    W_hh = np.ascontiguousarray(np.asarray(W_hh, dtype=np.float32))
    b_ih = np.ascontiguousarray(np.asarray(b_ih, dtype=np.float32))
    b_hh = np.ascontiguousarray(np.asarray(b_hh, dtype=np.float32))

    nc = _get_nc(x.shape[1])
    in_maps = [
        {"x": x[i * BS:(i + 1) * BS], "W_ih": W_ih, "W_hh": W_hh,
         "b_ih": b_ih, "b_hh": b_hh}
        for i in range(NCORES)
    ]
    res = run_bass_kernel_spmd(nc, in_maps, list(range(NCORES)), trace=_trace)
    outs = np.concatenate([res.results[i]["out"] for i in range(NCORES)], axis=0)
    h_t = np.concatenate([res.results[i]["h_out"] for i in range(NCORES)], axis=0)
    c_t = np.concatenate([res.results[i]["c_out"] for i in range(NCORES)], axis=0)
    if _trace:
        kernel._last_exec_time_ns = res.exec_time_ns
        kernel._last_results = res
    return outs, (h_t, c_t)
